# revision 1
# baseline (speedup 1.0000x reference)
"""nn_ColorReducer — Trainium2 Bass kernel (8-core data-parallel), v2.

Exact-enough nearest-palette-color via an index-encoding matmul trick:

  score_k(p) = 2 p.c_k - ||c_k||^2   (argmax_k <=> nearest color)

One matmul per 128-pixel tile computes BOTH
  cols   0..63 : s_k           (plain scores)
  cols 64..127 : s_k + k*DELTA (index-encoded scores)
where the k*DELTA row streams last in the contraction, so within a column
pair the f32 accumulation paths are identical except for one final add:
  max(cols 64..127) - max(cols 0..63) = argmax_index * DELTA +- ulp/2.
A single VectorE reduce-max over both halves + a tiny subtract yields
DELTA * index per pixel; the host rounds, clips, and gathers the palette.

Pixels/palette are split into 2 bf16 limbs (h + l); products ph*wh, pl*wh,
ph*wl kept (~1e-4 score accuracy; plenty for the 2e-2 rel-err gate).

Per 128-px tile the 12 contraction rows (partitions 0-11) are
  [ph_r ph_g ph_b  pl_r pl_g pl_b  ph_r ph_g ph_b  1  1  1]
paired with palette-side moving rows
  [wh_r wh_g wh_b  wh_r wh_g wh_b  wl_r wl_g wl_b  bh bl (0|k*DELTA)].

Sharding: batch dim (8 images) across the 8 NeuronCores; palette replicated.
"""

import numpy as np
import ml_dtypes

bf16 = ml_dtypes.bfloat16

B, C, H, W = 8, 3, 512, 512
HW = H * W                 # 262144 pixels per core
NCOL = 64                  # palette entries
KROWS = 12                 # contraction rows per tile
PXT = 128                  # pixels per tile (stationary columns)
NTILES = HW // PXT         # 2048
SG_TILES = 16              # tiles per PSUM supergroup (4 banks)
NSG = NTILES // SG_TILES   # 128 supergroups
SLAB_TILES = 128           # tiles per input DMA slab (32 KB/partition)
NSLABS = NTILES // SLAB_TILES  # 16
DELTA = float(2.0 ** -19)  # index encoding step

_CACHE = {}


def _split2(x):
    """f32 -> two bf16 limbs (h, l), h+l ~ x to ~2^-17 rel."""
    x = np.asarray(x, np.float32)
    h = x.astype(bf16)
    l = (x - h.astype(np.float32)).astype(bf16)
    return h, l


def _host_pixel_limbs(px):
    """px: (3, HW) f32 -> (12, HW) bf16 stationary rows."""
    ph, pl = _split2(px)
    rows = np.empty((KROWS, HW), dtype=bf16)
    rows[0:3] = ph
    rows[3:6] = pl
    rows[6:9] = ph
    rows[9:12] = bf16(1.0)
    return rows


def _host_palette_rows(palette):
    """palette: (64, 3) f32 -> (12, 128) bf16 moving rows.

    Col layout: k in 0..63 plain scores, 64+k index-encoded scores.
    """
    pal = np.asarray(palette, np.float64)
    w = (2.0 * pal).astype(np.float32)            # (64, 3)
    wh, wl = _split2(w)
    b = -(pal ** 2).sum(axis=1)                   # f64
    bh = b.astype(np.float32).astype(bf16)
    bl = (b - bh.astype(np.float64)).astype(np.float32).astype(bf16)
    rows = np.zeros((KROWS, 2 * NCOL), dtype=bf16)
    for c in range(3):
        for h in range(2):
            sl = slice(h * NCOL, (h + 1) * NCOL)
            rows[c, sl] = wh[:, c]
            rows[3 + c, sl] = wh[:, c]
            rows[6 + c, sl] = wl[:, c]
    rows[9, 0:NCOL] = bh
    rows[9, NCOL:] = bh
    rows[10, 0:NCOL] = bl
    rows[10, NCOL:] = bl
    rows[11, 0:NCOL] = bf16(0.0)
    rows[11, NCOL:] = (np.arange(NCOL, dtype=np.float32) * DELTA).astype(bf16)
    return np.ascontiguousarray(rows)


def _build_body(nc, tc, ctx, aps):
    import concourse.mybir as mybir

    f32 = mybir.dt.float32
    bft = mybir.dt.bfloat16

    consts = ctx.enter_context(tc.tile_pool(name="consts", bufs=1))
    slab_pool = ctx.enter_context(tc.tile_pool(name="slab", bufs=2))
    psum_pool = ctx.enter_context(tc.tile_pool(name="psum", bufs=2, space="PSUM"))
    max_pool = ctx.enter_context(tc.tile_pool(name="mx", bufs=1))
    cp_pool = ctx.enter_context(tc.tile_pool(name="cp", bufs=2))

    palT = consts.tile([KROWS, 2 * NCOL], bft)
    nc.sync.dma_start(palT[:], aps["palT"])

    # per-pixel (m0, m1) pairs for every tile; host does m1-m0 and decode
    maxes = max_pool.tile([PXT, NTILES * 2], f32)

    slab = None
    sg_per_slab = SLAB_TILES // SG_TILES  # 8
    for s in range(NSG):
        if s % sg_per_slab == 0:
            sl = s // sg_per_slab
            slab = slab_pool.tile([KROWS, SLAB_TILES * PXT], bft)
            nc.sync.dma_start(
                slab[:],
                aps["limbs"][:, sl * SLAB_TILES * PXT:(sl + 1) * SLAB_TILES * PXT],
            )
        ps = psum_pool.tile([PXT, SG_TILES * 2 * NCOL], f32)
        for u in range(SG_TILES):
            t_rel = (s % sg_per_slab) * SG_TILES + u
            nc.tensor.matmul(
                ps[:, u * 2 * NCOL:(u + 1) * 2 * NCOL],
                slab[:, t_rel * PXT:(t_rel + 1) * PXT],
                palT[:],
                start=True,
                stop=True,
            )
        if s % 2 == 0:
            cp = cp_pool.tile([PXT, 2 * SG_TILES * 2 * NCOL], f32)
        half = (s % 2) * SG_TILES * 2 * NCOL
        nc.scalar.copy(cp[:, half:half + SG_TILES * 2 * NCOL], ps[:])
        if s % 2 == 1:
            nc.vector.tensor_reduce(
                maxes[:, (s - 1) * SG_TILES * 2:(s + 1) * SG_TILES * 2].rearrange(
                    "p (u h) -> p u h", h=2
                ),
                cp[:].rearrange("p (u h k) -> p u h k", h=2, k=NCOL),
                axis=mybir.AxisListType.X,
                op=mybir.AluOpType.max,
            )
    nc.sync.dma_start(aps["maxes"], maxes[:])


def _build_nc():
    import concourse.mybir as mybir
    import concourse.tile as tile
    from concourse import bacc
    from contextlib import ExitStack

    nc = bacc.Bacc("TRN2", num_devices=8)
    aps = {
        "limbs": nc.dram_tensor(
            "limbs", (KROWS, HW), mybir.dt.bfloat16, kind="ExternalInput"
        ).ap(),
        "palT": nc.dram_tensor(
            "palT", (KROWS, 2 * NCOL), mybir.dt.bfloat16, kind="ExternalInput"
        ).ap(),
        "maxes": nc.dram_tensor(
            "maxes", (PXT, NTILES * 2), mybir.dt.float32, kind="ExternalOutput"
        ).ap(),
    }
    with tile.TileContext(nc) as tc:
        with ExitStack() as ctx:
            _build_body(nc, tc, ctx, aps)
    nc.compile()
    return nc


def _get_nc():
    if "nc" not in _CACHE:
        _CACHE["nc"] = _build_nc()
    return _CACHE["nc"]


def _host_inputs(x, palette):
    palT = _host_palette_rows(palette)
    in_maps = []
    for b in range(B):
        px = np.asarray(x[b], np.float32).reshape(3, HW)
        in_maps.append({"limbs": _host_pixel_limbs(px), "palT": palT})
    return in_maps


def _host_finish(maxes_list, palette, x):
    pal = np.asarray(palette, np.float32)
    pal64 = np.asarray(palette, np.float64)
    w64 = 2.0 * pal64                                 # (64, 3)
    b64 = -(pal64 ** 2).sum(axis=1)                   # (64,)
    out = np.empty((B, 3, H, W), np.float32)
    for b, mx in enumerate(maxes_list):
        mx = np.asarray(mx, np.float32).reshape(PXT, NTILES, 2)
        m0 = mx[:, :, 0].T.reshape(HW)                # px = t'*128 + p
        d = (mx[:, :, 1] - mx[:, :, 0]).T.reshape(HW) * (1.0 / DELTA)
        r = np.rint(d)
        idx = np.clip(r, 0, NCOL - 1).astype(np.int64)
        # Near-ties can make the encoded half's winner differ from the plain
        # half's, leaving a phantom in-between index. Flag pixels whose
        # residual is non-integer OR whose decoded color scores measurably
        # below the device max m0, and recompute their argmin exactly.
        px = np.asarray(x[b], np.float64).reshape(3, HW).T  # (HW, 3)
        s_true = (px * w64[idx]).sum(1) + b64[idx]
        suspect = (
            (np.abs(d - r) > 0.02)
            | (d < -0.02)
            | (d > NCOL - 1 + 0.02)
            | (m0 - s_true.astype(np.float32) > 1e-4)
        )
        lin = idx
        sus = np.where(suspect)[0]
        if sus.size:
            dist = ((px[sus, None, :] - pal64[None, :, :]) ** 2).sum(-1)
            lin[sus] = dist.argmin(1)
        out[b] = pal[lin].T.reshape(3, H, W)
    return out


def kernel(x, palette):
    from concourse.bass_utils import run_bass_kernel_spmd

    nc = _get_nc()
    in_maps = _host_inputs(x, palette)
    res = run_bass_kernel_spmd(nc, in_maps, core_ids=list(range(8)))
    maxes = [res.results[i]["maxes"] for i in range(B)]
    _CACHE["last_results"] = res
    return _host_finish(maxes, palette, x).astype(np.float32)



# revision 4
# speedup vs baseline: 1.9337x; 1.9337x over previous
"""nn_ColorReducer — Trainium2 Bass kernel (8-core data-parallel), v3.

Group-max scheme (no index-bit encoding):

  score_k(p) = 2 p.c_k - ||c_k||^2   (argmax_k <=> nearest color)

Per 128-pixel tile the PE computes the 64 plain scores (bf16 2-limb
arithmetic, ~1e-4 absolute accuracy). The 64 palette entries are split
into 8 groups of 8; the device reduces each group to its max in two
stages — GPSIMD tensor_tensor max (8 -> 4, straight from PSUM) then a
VectorE tensor_reduce (4 -> 1) — and streams the per-pixel 8 group
maxes to DRAM. Splitting the reduction across both engines roughly
halves the vector-side time vs a single tensor_reduce over 64 columns,
which is what bounds this kernel.

Host finish: winner group g* = argmax_g E_g, exact within-group argmax
over its 8 candidates (cheap: 8 dot products per pixel in numpy), and
pixels whose top-2 group maxes are within a small threshold (possible
cross-group near-tie at device precision) get an exact 64-way argmin.

Pixels/palette are split into 2 bf16 limbs (h + l); products ph*wh,
pl*wh, ph*wl kept. Contraction rows (partitions 0-11) per tile:
  stationary [ph_r ph_g ph_b  pl_r pl_g pl_b  ph_r ph_g ph_b  1 1 1]
  moving     [wh_r wh_g wh_b  wh_r wh_g wh_b  wl_r wl_g wl_b  bh bl 0]

Sharding: batch dim (8 images) across the 8 NeuronCores; palette
replicated.
"""

import numpy as np
import ml_dtypes

bf16 = ml_dtypes.bfloat16

B, C, H, W = 8, 3, 512, 512
HW = H * W                 # 262144 pixels per core
NCOL = 64                  # palette entries
NGRP = 8                   # score groups per pixel
GSZ = NCOL // NGRP         # 8 colors per group
KROWS = 12                 # contraction rows per tile
PXT = 128                  # pixels per tile (stationary columns)
NTILES = HW // PXT         # 2048
SG_TILES = 32              # tiles per PSUM supergroup (4 banks)
NSG = NTILES // SG_TILES   # 64
SLAB_TILES = 128           # tiles per input DMA slab (32 KB/partition)
NSLABS = NTILES // SLAB_TILES  # 16
OUT_CHUNK_SG = 8           # supergroups per output DMA chunk

_CACHE = {}


def _split2(x):
    """f32 -> two bf16 limbs (h, l), h+l ~ x to ~2^-17 rel."""
    x = np.asarray(x, np.float32)
    h = x.astype(bf16)
    l = (x - h.astype(np.float32)).astype(bf16)
    return h, l


def _host_pixel_limbs(px):
    """px: (3, HW) f32 -> (12, HW) bf16 stationary rows."""
    ph, pl = _split2(px)
    rows = np.empty((KROWS, HW), dtype=bf16)
    rows[0:3] = ph
    rows[3:6] = pl
    rows[6:9] = ph
    rows[9:12] = bf16(1.0)
    return rows


def _host_palette_rows(palette):
    """palette: (64, 3) f32 -> (12, 64) bf16 moving rows (plain scores)."""
    pal = np.asarray(palette, np.float64)
    w = (2.0 * pal).astype(np.float32)            # (64, 3)
    wh, wl = _split2(w)
    b = -(pal ** 2).sum(axis=1)                   # f64
    bh = b.astype(np.float32).astype(bf16)
    bl = (b - bh.astype(np.float64)).astype(np.float32).astype(bf16)
    rows = np.zeros((KROWS, NCOL), dtype=bf16)
    for c in range(3):
        rows[c] = wh[:, c]
        rows[3 + c] = wh[:, c]
        rows[6 + c] = wl[:, c]
    rows[9] = bh
    rows[10] = bl
    # row 11 stays zero (pairs with the ones row; keeps KROWS layout)
    return np.ascontiguousarray(rows)


def _build_body(nc, tc, ctx, aps):
    import concourse.mybir as mybir

    f32 = mybir.dt.float32
    bft = mybir.dt.bfloat16

    consts = ctx.enter_context(tc.tile_pool(name="consts", bufs=1))
    slab_pool = ctx.enter_context(tc.tile_pool(name="slab", bufs=2))
    psum_pool = ctx.enter_context(tc.tile_pool(name="psum", bufs=2, space="PSUM"))
    cp_pool = ctx.enter_context(tc.tile_pool(name="cp", bufs=3))
    l1_pool = ctx.enter_context(tc.tile_pool(name="l1", bufs=3))
    l2_pool = ctx.enter_context(tc.tile_pool(name="l2", bufs=3))
    mx_pool = ctx.enter_context(tc.tile_pool(name="mx", bufs=2))

    palT = consts.tile([KROWS, NCOL], bft)
    nc.sync.dma_start(palT[:], aps["palT"])

    J2 = GSZ // 2  # 4
    sg_per_slab = SLAB_TILES // SG_TILES
    slab = None
    mx = None
    for s in range(NSG):
        if s % sg_per_slab == 0:
            sl = s // sg_per_slab
            slab = slab_pool.tile([KROWS, SLAB_TILES * PXT], bft)
            nc.sync.dma_start(
                slab[:],
                aps["limbs"][:, sl * SLAB_TILES * PXT:(sl + 1) * SLAB_TILES * PXT],
            )
        if s % OUT_CHUNK_SG == 0:
            mx = mx_pool.tile([PXT, OUT_CHUNK_SG * SG_TILES * NGRP], f32)
        ps = psum_pool.tile([PXT, SG_TILES * NCOL], f32)
        for u in range(SG_TILES):
            t_rel = (s % sg_per_slab) * SG_TILES + u
            nc.tensor.matmul(
                ps[:, u * NCOL:(u + 1) * NCOL],
                slab[:, t_rel * PXT:(t_rel + 1) * PXT],
                palT[:],
                start=True,
                stop=True,
            )
        # scores per tile: 8 groups x 8 entries (g-major). Max over each
        # group via a 3-level pairwise-max chain; the walrus verifier bans
        # two-PSUM-operand TensorTensor and any GPSIMD PSUM access, so ACT
        # first copies the j>=4 half into SBUF, then DVE runs the chain.
        v = ps[:].rearrange(
            "p (u g two j) -> p u g two j", g=NGRP, two=2, j=J2
        )
        cp = cp_pool.tile([PXT, SG_TILES * NGRP * J2], f32)
        cpv = cp[:].rearrange("p (u g j) -> p u g j", g=NGRP, j=J2)
        nc.scalar.copy(cpv, v[:, :, :, 1, :])
        l1 = l1_pool.tile([PXT, SG_TILES * NGRP * J2], f32)
        nc.vector.tensor_tensor(
            l1[:].rearrange("p (u g j) -> p u g j", g=NGRP, j=J2),
            v[:, :, :, 0, :],
            cpv,
            op=mybir.AluOpType.max,
        )
        w2 = l1[:].rearrange("p (u g two j) -> p u g two j", g=NGRP, two=2, j=2)
        l2 = l2_pool.tile([PXT, SG_TILES * NGRP * 2], f32)
        nc.vector.tensor_tensor(
            l2[:].rearrange("p (u g j) -> p u g j", g=NGRP, j=2),
            w2[:, :, :, 0, :],
            w2[:, :, :, 1, :],
            op=mybir.AluOpType.max,
        )
        off = (s % OUT_CHUNK_SG) * SG_TILES * NGRP
        w3 = l2[:].rearrange("p (u g two) -> p u g two", g=NGRP, two=2)
        nc.vector.tensor_tensor(
            mx[:, off:off + SG_TILES * NGRP].rearrange(
                "p (u g x) -> p u g x", g=NGRP, x=1
            ),
            w3[:, :, :, 0:1],
            w3[:, :, :, 1:2],
            op=mybir.AluOpType.max,
        )
        if s % OUT_CHUNK_SG == OUT_CHUNK_SG - 1:
            c0 = (s - (OUT_CHUNK_SG - 1)) * SG_TILES * NGRP
            nc.sync.dma_start(
                aps["grp"][:, c0:c0 + OUT_CHUNK_SG * SG_TILES * NGRP], mx[:]
            )


def _build_nc():
    import concourse.mybir as mybir
    import concourse.tile as tile
    from concourse import bacc
    from contextlib import ExitStack

    nc = bacc.Bacc("TRN2", num_devices=8)
    aps = {
        "limbs": nc.dram_tensor(
            "limbs", (KROWS, HW), mybir.dt.bfloat16, kind="ExternalInput"
        ).ap(),
        "palT": nc.dram_tensor(
            "palT", (KROWS, NCOL), mybir.dt.bfloat16, kind="ExternalInput"
        ).ap(),
        "grp": nc.dram_tensor(
            "grp", (PXT, NTILES * NGRP), mybir.dt.float32, kind="ExternalOutput"
        ).ap(),
    }
    with tile.TileContext(nc) as tc:
        with ExitStack() as ctx:
            _build_body(nc, tc, ctx, aps)
    nc.compile()
    return nc


def _get_nc():
    if "nc" not in _CACHE:
        _CACHE["nc"] = _build_nc()
    return _CACHE["nc"]


def _host_inputs(x, palette):
    palT = _host_palette_rows(palette)
    in_maps = []
    for b in range(B):
        px = np.asarray(x[b], np.float32).reshape(3, HW)
        in_maps.append({"limbs": _host_pixel_limbs(px), "palT": palT})
    return in_maps


# flag a pixel for exact recompute when its top-2 device group maxes are
# closer than this (possible cross-group flip at device precision ~1e-4)
TIE_TAU = 6e-4


def _host_finish(grp_list, palette, x):
    pal = np.asarray(palette, np.float32)
    pal64 = np.asarray(palette, np.float64)
    w64 = 2.0 * pal64                                 # (64, 3)
    b64 = -(pal64 ** 2).sum(axis=1)                   # (64,)
    out = np.empty((B, 3, H, W), np.float32)
    for b, g in enumerate(grp_list):
        # g: (PXT, NTILES*NGRP) -> E[pixel, group]; pixel = t*128 + p
        E = (
            np.asarray(g, np.float32)
            .reshape(PXT, NTILES, NGRP)
            .transpose(1, 0, 2)
            .reshape(HW, NGRP)
        )
        px = np.asarray(x[b], np.float64).reshape(3, HW).T   # (HW, 3)
        order = np.argsort(E, axis=1)
        gstar = order[:, -1]
        margin = E[np.arange(HW), gstar] - E[np.arange(HW), order[:, -2]]
        # exact scores within the winning group (8 candidates)
        cand = gstar[:, None] * GSZ + np.arange(GSZ)[None, :]   # (HW, 8)
        s = np.einsum("pc,pjc->pj", px, w64[cand]) + b64[cand]
        idx = cand[np.arange(HW), s.argmax(1)]
        # cross-group near-ties: exact 64-way argmin
        sus = np.where(margin < TIE_TAU)[0]
        if sus.size:
            dist = ((px[sus, None, :] - pal64[None, :, :]) ** 2).sum(-1)
            idx[sus] = dist.argmin(1)
        out[b] = pal[idx].T.reshape(3, H, W)
    return out


def kernel(x, palette):
    from concourse.bass_utils import run_bass_kernel_spmd

    nc = _get_nc()
    in_maps = _host_inputs(x, palette)
    res = run_bass_kernel_spmd(nc, in_maps, core_ids=list(range(8)))
    grp = [res.results[i]["grp"] for i in range(B)]
    _CACHE["last_results"] = res
    return _host_finish(grp, palette, x).astype(np.float32)
